# revision 34
# baseline (speedup 1.0000x reference)
"""Channel-attention (CAM) Trainium2 kernel.

Problem: out[b] = softmax(b_f[b] @ c_f[b].T, axis=-1) @ a_f[b] + a_f[b]
with a,b,c: [16, 1024, 32, 32] fp32, flattened to [16, 1024, 1024].

Sharding: pure data parallel over batch — 16 samples / 8 cores = 2 per core.

Per-core pipeline (per sample), fp16 compute:
  - b,c cast fp32->fp16 via DRAM->DRAM cast DMA, then DMA-transposed
    (XBAR) into [HW, C] operand layout (bT, cT)
  - m1: scores = bT.T @ cT, fp16 x1 (or x3 hi/lo split for high accuracy),
    fp32 PSUM accumulation
  - softmax: DVE row-max along free dim, ACT Exp with bias=-max and
    accum_out row-sum; the 1/sum division is deferred to the output
  - E (fp16) PE-transposed into ET (stationary operand of m2)
  - m2: out = ET.T @ a16, fp32 PSUM accumulation
  - finalize: one DVE scalar_tensor_tensor: out = psum * (1/sum) + a_fp32

Note: PE never executes fp32 ops — fp32 transpose-mode matmuls were
observed to hang the PE intermittently when interleaved with 16-bit
FWL-eligible matmul streams.
"""
import os
import sys
import types

import numpy as np


def _install_axon_hooks():
    """Provide antenv.axon_hooks (missing in this image) so trace=True works."""
    if 'antenv.axon_hooks' in sys.modules:
        return
    m = types.ModuleType('antenv.axon_hooks')
    m._hook = None
    m.set_axon_ntff_profile_hook = lambda h: setattr(m, '_hook', h)
    m.get_axon_ntff_profile_hook = lambda: m._hook
    sys.modules['antenv.axon_hooks'] = m
    try:
        import antenv
        antenv.axon_hooks = m
    except ImportError:
        pass
    try:
        from trn_agent_boot.trn_boot import _ntff_profile_via_ctypes
        m.set_axon_ntff_profile_hook(
            _ntff_profile_via_ctypes('/opt/axon/libaxon_pjrt.so'))
    except Exception:
        pass


_install_axon_hooks()

import concourse.bass as bass  # noqa: E402
import concourse.mybir as mybir  # noqa: E402
import concourse.tile as tile  # noqa: E402
from concourse import bacc, bass_utils  # noqa: E402
from concourse.masks import make_identity  # noqa: E402
from concourse.tile_rust import add_dep_helper  # noqa: E402

# artifact upload needs a bucket; keep everything local in the sandbox
bass_utils.upload_artifacts = lambda tmpdir: f"local:{tmpdir}"

N_CORES = 8
B, C, H, W = 16, 1024, 32, 32
HW = H * W
S = B // N_CORES        # samples per core
P = 128
NT = C // P             # 8 row tiles
F32 = mybir.dt.float32
F16 = mybir.dt.float16
ALU = mybir.AluOpType
AX = mybir.AxisListType
ACTF = mybir.ActivationFunctionType

# 1 = single-pass fp16 m1 (fast); 3 = fp16 hi/lo x3 m1 (high accuracy)
M1_TERMS = int(os.environ.get("CAM_M1_TERMS", "1"))


def cam_kernel(ctx, tc, out_ap, a_ap, b_ap, c_ap, n_samples=S):
    nc = tc.nc

    const_pool = ctx.enter_context(tc.tile_pool(name="const", bufs=1))
    big = ctx.enter_context(tc.tile_pool(name="big", bufs=1))
    big2 = ctx.enter_context(tc.tile_pool(name="big2", bufs=2))
    stg_pool = ctx.enter_context(tc.tile_pool(name="stage", bufs=3))
    epool = ctx.enter_context(tc.tile_pool(name="epool", bufs=2))
    opool = ctx.enter_context(tc.tile_pool(name="opool", bufs=2))
    arpool = ctx.enter_context(tc.tile_pool(name="ar", bufs=3))
    sm = ctx.enter_context(tc.tile_pool(name="sm", bufs=16))
    dram = ctx.enter_context(tc.tile_pool(name="dram", bufs=2, space="DRAM"))
    psum_t = ctx.enter_context(tc.tile_pool(name="psum_t", bufs=1, space="PSUM"))
    psum_s = ctx.enter_context(tc.tile_pool(name="psum_s", bufs=4, space="PSUM"))
    psum_o = ctx.enter_context(tc.tile_pool(name="psum_o", bufs=3, space="PSUM"))

    ident = const_pool.tile([P, P], F16)
    make_identity(nc, ident[:])

    for s in range(n_samples):
        a16 = big2.tile([P, NT, HW], F16, tag="a16")
        ET = big.tile([P, NT, C], F16, tag="ET")

        # ---- b,c -> fp16 transposed operands (merged staging tensor) ----
        # bcT free-dim layout (units of C): x1: [b, c]; x3: [b_hi, b_lo,
        # c_hi, c_lo]. m1_ops = (lhs col base, rhs col base) pairs.
        if M1_TERMS == 1:
            NW = 2
            bcT = big2.tile([P, NT, NW * C], F16, tag="bcT")
            bc16d = dram.tile([NW * C, HW], F16, tag="bc16d")
            # column-half casts so the first transposes unlock early
            for q in range(2):
                qs = slice(q * 512, (q + 1) * 512)
                nc.gpsimd.dma_start(bc16d[0:C, qs], b_ap[s, :, qs])
                nc.gpsimd.dma_start(bc16d[C:2 * C, qs], c_ap[s, :, qs])
            m1_ops = [(0, C)]
        else:
            NW = 4
            bcT = big.tile([P, NT, NW * C], F16, tag="bcT3")
            bc16d = dram.tile([NW * C, HW], F16, tag="bc16d3")
            for src_ap, base in ((b_ap, 0), (c_ap, 2 * C)):
                for r in range(NT):
                    st = stg_pool.tile([P, HW], F32, tag="stage")
                    nc.sync.dma_start(st[:], src_ap[s, r * P:(r + 1) * P, :])
                    hi_nat = stg_pool.tile([P, HW], F16, tag="hi_nat")
                    lo_nat = stg_pool.tile([P, HW], F16, tag="lo_nat")
                    nc.vector.tensor_copy(hi_nat[:], st[:])
                    nc.vector.tensor_tensor(lo_nat[:], st[:], hi_nat[:],
                                            ALU.subtract)
                    rsl = slice(r * P, (r + 1) * P)
                    nc.scalar.dma_start(bc16d[base:base + C, :][rsl, :],
                                        hi_nat[:])
                    nc.scalar.dma_start(bc16d[base + C:base + 2 * C, :][rsl, :],
                                        lo_nat[:])
            m1_ops = [(0, 2 * C), (0, 3 * C), (C, 2 * C)]
        prev_tr = None
        for j in range(NT):
            h = nc.sync.dma_start(bcT[:, j, :], bc16d[:, j * P:(j + 1) * P],
                                  transpose=True)
            if prev_tr is not None:
                add_dep_helper(h.ins, prev_tr.ins, sync=False,
                               reason="batch xbar transposes")
            prev_tr = h

        # ---- a -> fp16 via cast-DMA (m2 moving operand) ----
        for r in range(NT):
            nc.gpsimd.dma_start(a16[:, r, :], a_ap[s, r * P:(r + 1) * P, :])

        # ---- per output row-tile: m1, softmax, E^T, m2, finalize ----
        for i in range(NT):
            isl = slice(i * P, (i + 1) * P)
            ps0 = psum_s.tile([P, 512], F32, tag="ps")
            ps1 = psum_s.tile([P, 512], F32, tag="ps")
            n_acc = NT * len(m1_ops)
            acc = 0
            for kk in range(NT):
                for lo_, ro_ in m1_ops:
                    first = acc == 0
                    last = acc == n_acc - 1
                    lhsT = bcT[:, kk, lo_ + i * P:lo_ + (i + 1) * P]
                    nc.tensor.matmul(ps0[:], lhsT, bcT[:, kk, ro_:ro_ + 512],
                                     start=first, stop=last)
                    nc.tensor.matmul(ps1[:], lhsT,
                                     bcT[:, kk, ro_ + 512:ro_ + 1024],
                                     start=first, stop=last)
                    acc += 1

            m0 = sm.tile([P, 1], F32, tag="sc")
            m1t = sm.tile([P, 1], F32, tag="sc")
            nmx = sm.tile([P, 1], F32, tag="sc")
            nc.vector.tensor_reduce(m0[:], ps0[:], axis=AX.X, op=ALU.max)
            nc.vector.tensor_reduce(m1t[:], ps1[:], axis=AX.X, op=ALU.max)
            nc.vector.tensor_tensor(nmx[:], m0[:], m1t[:], ALU.max)
            nc.vector.tensor_scalar_mul(nmx[:], nmx[:], -1.0)

            E = epool.tile([P, C], F16, tag="E")
            rs0 = sm.tile([P, 1], F32, tag="sc")
            rs1 = sm.tile([P, 1], F32, tag="sc")
            nc.scalar.activation(E[:, 0:512], ps0[:], ACTF.Exp,
                                 bias=nmx[:], scale=1.0, accum_out=rs0[:])
            nc.scalar.activation(E[:, 512:1024], ps1[:], ACTF.Exp,
                                 bias=nmx[:], scale=1.0, accum_out=rs1[:])
            rinv = sm.tile([P, 1], F32, tag="sc")
            nc.vector.tensor_add(rinv[:], rs0[:], rs1[:])
            nc.vector.reciprocal(rinv[:], rinv[:])

            pt = psum_t.tile([P, 8 * P], F16, tag="pt")
            for j in range(8):
                nc.tensor.transpose(
                    pt[:, j * P:(j + 1) * P],
                    E[:, j * P:(j + 1) * P], ident[:])
            nc.vector.tensor_copy(
                ET[:, :, isl],
                pt[:].rearrange("p (t c) -> p t c", t=8))

            po0 = psum_o.tile([P, 512], F32, tag="po")
            po1 = psum_o.tile([P, 512], F32, tag="po")
            for jj in range(NT):
                first, last = jj == 0, jj == NT - 1
                l_e = ET[:, jj, isl]
                nc.tensor.matmul(po0[:], l_e, a16[:, jj, 0:512],
                                 start=first, stop=last)
                nc.tensor.matmul(po1[:], l_e, a16[:, jj, 512:1024],
                                 start=first, stop=last)

            ar = arpool.tile([P, HW], F32, tag="ar")
            nc.scalar.dma_start(ar[:], a_ap[s, isl, :])
            ot = opool.tile([P, HW], F32, tag="ot")
            nc.vector.scalar_tensor_tensor(
                ot[:, 0:512], po0[:], rinv[:], ar[:, 0:512],
                op0=ALU.mult, op1=ALU.add)
            nc.vector.scalar_tensor_tensor(
                ot[:, 512:1024], po1[:], rinv[:], ar[:, 512:1024],
                op0=ALU.mult, op1=ALU.add)
            nc.scalar.dma_start(out_ap[s, isl, :], ot[:])


_BUILT = {}


def build_program(n_samples=S):
    key = (M1_TERMS, n_samples)
    if key in _BUILT:
        return _BUILT[key]
    nc = bacc.Bacc("TRN2", target_bir_lowering=False, debug=False,
                   enable_asserts=False, num_devices=N_CORES)
    a = nc.dram_tensor("a", [S, C, HW], F32, kind="ExternalInput").ap()
    b = nc.dram_tensor("b", [S, C, HW], F32, kind="ExternalInput").ap()
    c = nc.dram_tensor("c", [S, C, HW], F32, kind="ExternalInput").ap()
    out = nc.dram_tensor("out", [S, C, HW], F32, kind="ExternalOutput").ap()
    from contextlib import ExitStack
    with tile.TileContext(nc) as tc, ExitStack() as ctx:
        cam_kernel(ctx, tc, out, a, b, c, n_samples=n_samples)
    nc.compile()
    _BUILT[key] = nc
    return nc


def run_sharded(a, b, c, trace=False, n_samples=S, **kw):
    """a,b,c: [16,1024,1024] fp32 -> (full output, BassKernelResults)."""
    nc = build_program(n_samples)
    in_maps = []
    for core in range(N_CORES):
        sl = slice(core * S, (core + 1) * S)
        in_maps.append({"a": np.ascontiguousarray(a[sl]),
                        "b": np.ascontiguousarray(b[sl]),
                        "c": np.ascontiguousarray(c[sl])})
    res = bass_utils.run_bass_kernel_spmd(
        nc, in_maps, core_ids=list(range(N_CORES)), trace=trace, **kw)
    out = np.concatenate([res.results[core]["out"] for core in range(N_CORES)],
                         axis=0)
    return out, res


def kernel(a, b, c):
    a = np.asarray(a, dtype=np.float32).reshape(B, C, HW)
    b = np.asarray(b, dtype=np.float32).reshape(B, C, HW)
    c = np.asarray(c, dtype=np.float32).reshape(B, C, HW)
    out, _ = run_sharded(a, b, c, trace=False)
    return out.reshape(B, C, H, W)


# revision 37
# speedup vs baseline: 1.3014x; 1.3014x over previous
"""Channel-attention (CAM) Trainium2 kernel.

Problem: out[b] = softmax(b_f[b] @ c_f[b].T, axis=-1) @ a_f[b] + a_f[b]
with a,b,c: [16, 1024, 32, 32] fp32, flattened to [16, 1024, 1024].

Sharding: pure data parallel over batch — 16 samples / 8 cores = 2 per core.

Per-core pipeline (per sample), fp16 compute:
  - b,c cast fp32->fp16 via DRAM->DRAM cast DMA, then DMA-transposed
    (XBAR) into [HW, C] operand layout (bT, cT)
  - m1: scores = bT.T @ cT, fp16 x1 (or x3 hi/lo split for high accuracy),
    fp32 PSUM accumulation
  - softmax: DVE row-max along free dim, ACT Exp with bias=-max and
    accum_out row-sum; the 1/sum division is deferred to the output
  - E (fp16) PE-transposed into ET (stationary operand of m2)
  - m2: out = ET.T @ a16, fp32 PSUM accumulation
  - finalize: one DVE scalar_tensor_tensor: out = psum * (1/sum) + a_fp32

Note: PE never executes fp32 ops — fp32 transpose-mode matmuls were
observed to hang the PE intermittently when interleaved with 16-bit
FWL-eligible matmul streams.
"""
import os
import sys
import types

import numpy as np


def _install_axon_hooks():
    """Provide antenv.axon_hooks (missing in this image) so trace=True works."""
    if 'antenv.axon_hooks' in sys.modules:
        return
    m = types.ModuleType('antenv.axon_hooks')
    m._hook = None
    m.set_axon_ntff_profile_hook = lambda h: setattr(m, '_hook', h)
    m.get_axon_ntff_profile_hook = lambda: m._hook
    sys.modules['antenv.axon_hooks'] = m
    try:
        import antenv
        antenv.axon_hooks = m
    except ImportError:
        pass
    try:
        from trn_agent_boot.trn_boot import _ntff_profile_via_ctypes
        m.set_axon_ntff_profile_hook(
            _ntff_profile_via_ctypes('/opt/axon/libaxon_pjrt.so'))
    except Exception:
        pass


_install_axon_hooks()

import concourse.bass as bass  # noqa: E402
import concourse.mybir as mybir  # noqa: E402
import concourse.tile as tile  # noqa: E402
from concourse import bacc, bass_utils  # noqa: E402
from concourse.masks import make_identity  # noqa: E402
from concourse.tile_rust import add_dep_helper  # noqa: E402

# artifact upload needs a bucket; keep everything local in the sandbox
bass_utils.upload_artifacts = lambda tmpdir: f"local:{tmpdir}"

N_CORES = 8
B, C, H, W = 16, 1024, 32, 32
HW = H * W
S = B // N_CORES        # samples per core
P = 128
NT = C // P             # 8 row tiles
F32 = mybir.dt.float32
F16 = mybir.dt.float16
ALU = mybir.AluOpType
AX = mybir.AxisListType
ACTF = mybir.ActivationFunctionType

# 1 = single-pass fp16 m1 (fast); 3 = fp16 hi/lo x3 m1 (high accuracy)
M1_TERMS = int(os.environ.get("CAM_M1_TERMS", "1"))


def cam_kernel(ctx, tc, out_ap, a_ap, b_ap, c_ap, n_samples=S):
    nc = tc.nc

    const_pool = ctx.enter_context(tc.tile_pool(name="const", bufs=1))
    big = ctx.enter_context(tc.tile_pool(name="big", bufs=1))
    big2 = ctx.enter_context(tc.tile_pool(name="big2", bufs=2))
    stg_pool = ctx.enter_context(tc.tile_pool(name="stage", bufs=3))
    epool = ctx.enter_context(tc.tile_pool(name="epool", bufs=2))
    opool = ctx.enter_context(tc.tile_pool(name="opool", bufs=2))
    arpool = ctx.enter_context(tc.tile_pool(name="ar", bufs=3))
    sm = ctx.enter_context(tc.tile_pool(name="sm", bufs=16))
    dram = ctx.enter_context(tc.tile_pool(name="dram", bufs=2, space="DRAM"))
    psum_t = ctx.enter_context(tc.tile_pool(name="psum_t", bufs=2, space="PSUM"))
    psum_s = ctx.enter_context(tc.tile_pool(name="psum_s", bufs=3, space="PSUM"))
    psum_o = ctx.enter_context(tc.tile_pool(name="psum_o", bufs=3, space="PSUM"))

    ident = const_pool.tile([P, P], F16)
    make_identity(nc, ident[:])

    for s in range(n_samples):
        a16 = big2.tile([P, NT, HW], F16, tag="a16")
        ET = big.tile([P, NT, C], F16, tag="ET")

        # ---- b,c -> fp16 transposed operands via cast-load + PE transpose ----
        # bcT free-dim layout (units of C): x1: [b, c]; x3: [b_hi, b_lo,
        # c_hi, c_lo]. m1_ops = (lhs col base, rhs col base) pairs.
        if M1_TERMS == 1:
            NW = 2
            bcT = big2.tile([P, NT, NW * C], F16, tag="bcT")
            srcs = [(b_ap, 0, None), (c_ap, C, None)]
            m1_ops = [(0, C)]
        else:
            NW = 4
            bcT = big.tile([P, NT, NW * C], F16, tag="bcT3")
            srcs = [(b_ap, 0, 1), (c_ap, 2 * C, 3)]
            m1_ops = [(0, 2 * C), (0, 3 * C), (C, 2 * C)]
        for src_ap, base, lo_q in srcs:
            for r in range(NT):
                rsl = slice(r * P, (r + 1) * P)
                nat = stg_pool.tile([P, HW], F16, tag="nat")
                nc.gpsimd.dma_start(nat[:], src_ap[s, rsl, :])  # cast f32->f16
                if lo_q is not None:
                    st = stg_pool.tile([P, HW], F32, tag="stage")
                    nc.sync.dma_start(st[:], src_ap[s, rsl, :])
                    lon = stg_pool.tile([P, HW], F16, tag="lon")
                    nc.vector.tensor_tensor(lon[:], st[:], nat[:],
                                            ALU.subtract)
                    nats = ((nat, base), (lon, base + C))
                else:
                    nats = ((nat, base),)
                for nt_, nb in nats:
                    pt = psum_t.tile([P, 8 * P], F16, tag="pt")
                    for j in range(NT):
                        nc.tensor.transpose(
                            pt[:, j * P:(j + 1) * P],
                            nt_[:, j * P:(j + 1) * P], ident[:])
                    nc.vector.tensor_copy(
                        bcT[:, :, nb + r * P:nb + (r + 1) * P],
                        pt[:].rearrange("p (t c) -> p t c", t=8))

        # ---- a -> fp16 via cast-DMA (m2 moving operand) ----
        for r in range(NT):
            nc.gpsimd.dma_start(a16[:, r, :], a_ap[s, r * P:(r + 1) * P, :])

        # ---- per output row-tile: m1, softmax, E^T, m2, finalize ----
        for i in range(NT):
            isl = slice(i * P, (i + 1) * P)
            ps0 = psum_s.tile([P, 512], F32, tag="ps")
            ps1 = psum_s.tile([P, 512], F32, tag="ps")
            n_acc = NT * len(m1_ops)
            acc = 0
            for kk in range(NT):
                for lo_, ro_ in m1_ops:
                    first = acc == 0
                    last = acc == n_acc - 1
                    lhsT = bcT[:, kk, lo_ + i * P:lo_ + (i + 1) * P]
                    nc.tensor.matmul(ps0[:], lhsT, bcT[:, kk, ro_:ro_ + 512],
                                     start=first, stop=last)
                    nc.tensor.matmul(ps1[:], lhsT,
                                     bcT[:, kk, ro_ + 512:ro_ + 1024],
                                     start=first, stop=last)
                    acc += 1

            m0 = sm.tile([P, 1], F32, tag="sc")
            m1t = sm.tile([P, 1], F32, tag="sc")
            nmx = sm.tile([P, 1], F32, tag="sc")
            nc.vector.tensor_reduce(m0[:], ps0[:], axis=AX.X, op=ALU.max)
            nc.vector.tensor_reduce(m1t[:], ps1[:], axis=AX.X, op=ALU.max)
            nc.vector.tensor_tensor(nmx[:], m0[:], m1t[:], ALU.max)
            nc.vector.tensor_scalar_mul(nmx[:], nmx[:], -1.0)

            E = epool.tile([P, C], F16, tag="E")
            rs0 = sm.tile([P, 1], F32, tag="sc")
            rs1 = sm.tile([P, 1], F32, tag="sc")
            nc.scalar.activation(E[:, 0:512], ps0[:], ACTF.Exp,
                                 bias=nmx[:], scale=1.0, accum_out=rs0[:])
            nc.scalar.activation(E[:, 512:1024], ps1[:], ACTF.Exp,
                                 bias=nmx[:], scale=1.0, accum_out=rs1[:])
            rinv = sm.tile([P, 1], F32, tag="sc")
            nc.vector.tensor_add(rinv[:], rs0[:], rs1[:])
            nc.vector.reciprocal(rinv[:], rinv[:])

            pt = psum_t.tile([P, 8 * P], F16, tag="pt")
            for j in range(8):
                nc.tensor.transpose(
                    pt[:, j * P:(j + 1) * P],
                    E[:, j * P:(j + 1) * P], ident[:])
            nc.vector.tensor_copy(
                ET[:, :, isl],
                pt[:].rearrange("p (t c) -> p t c", t=8))

            po0 = psum_o.tile([P, 512], F32, tag="po")
            po1 = psum_o.tile([P, 512], F32, tag="po")
            for jj in range(NT):
                first, last = jj == 0, jj == NT - 1
                l_e = ET[:, jj, isl]
                nc.tensor.matmul(po0[:], l_e, a16[:, jj, 0:512],
                                 start=first, stop=last)
                nc.tensor.matmul(po1[:], l_e, a16[:, jj, 512:1024],
                                 start=first, stop=last)

            ar = arpool.tile([P, HW], F32, tag="ar")
            nc.scalar.dma_start(ar[:], a_ap[s, isl, :])
            ot = opool.tile([P, HW], F32, tag="ot")
            nc.vector.scalar_tensor_tensor(
                ot[:, 0:512], po0[:], rinv[:], ar[:, 0:512],
                op0=ALU.mult, op1=ALU.add)
            nc.vector.scalar_tensor_tensor(
                ot[:, 512:1024], po1[:], rinv[:], ar[:, 512:1024],
                op0=ALU.mult, op1=ALU.add)
            nc.scalar.dma_start(out_ap[s, isl, :], ot[:])


_BUILT = {}


def build_program(n_samples=S):
    key = (M1_TERMS, n_samples)
    if key in _BUILT:
        return _BUILT[key]
    nc = bacc.Bacc("TRN2", target_bir_lowering=False, debug=False,
                   enable_asserts=False, num_devices=N_CORES)
    a = nc.dram_tensor("a", [S, C, HW], F32, kind="ExternalInput").ap()
    b = nc.dram_tensor("b", [S, C, HW], F32, kind="ExternalInput").ap()
    c = nc.dram_tensor("c", [S, C, HW], F32, kind="ExternalInput").ap()
    out = nc.dram_tensor("out", [S, C, HW], F32, kind="ExternalOutput").ap()
    from contextlib import ExitStack
    with tile.TileContext(nc) as tc, ExitStack() as ctx:
        cam_kernel(ctx, tc, out, a, b, c, n_samples=n_samples)
    nc.compile()
    _BUILT[key] = nc
    return nc


def run_sharded(a, b, c, trace=False, n_samples=S, **kw):
    """a,b,c: [16,1024,1024] fp32 -> (full output, BassKernelResults)."""
    nc = build_program(n_samples)
    in_maps = []
    for core in range(N_CORES):
        sl = slice(core * S, (core + 1) * S)
        in_maps.append({"a": np.ascontiguousarray(a[sl]),
                        "b": np.ascontiguousarray(b[sl]),
                        "c": np.ascontiguousarray(c[sl])})
    res = bass_utils.run_bass_kernel_spmd(
        nc, in_maps, core_ids=list(range(N_CORES)), trace=trace, **kw)
    out = np.concatenate([res.results[core]["out"] for core in range(N_CORES)],
                         axis=0)
    return out, res


def kernel(a, b, c):
    a = np.asarray(a, dtype=np.float32).reshape(B, C, HW)
    b = np.asarray(b, dtype=np.float32).reshape(B, C, HW)
    c = np.asarray(c, dtype=np.float32).reshape(B, C, HW)
    out, _ = run_sharded(a, b, c, trace=False)
    return out.reshape(B, C, H, W)


# revision 38
# speedup vs baseline: 1.3405x; 1.0300x over previous
"""Channel-attention (CAM) Trainium2 kernel.

Problem: out[b] = softmax(b_f[b] @ c_f[b].T, axis=-1) @ a_f[b] + a_f[b]
with a,b,c: [16, 1024, 32, 32] fp32, flattened to [16, 1024, 1024].

Sharding: pure data parallel over batch — 16 samples / 8 cores = 2 per core.

Per-core pipeline (per sample), fp16 compute:
  - b,c cast fp32->fp16 via DRAM->DRAM cast DMA, then DMA-transposed
    (XBAR) into [HW, C] operand layout (bT, cT)
  - m1: scores = bT.T @ cT, fp16 x1 (or x3 hi/lo split for high accuracy),
    fp32 PSUM accumulation
  - softmax: DVE row-max along free dim, ACT Exp with bias=-max and
    accum_out row-sum; the 1/sum division is deferred to the output
  - E (fp16) PE-transposed into ET (stationary operand of m2)
  - m2: out = ET.T @ a16, fp32 PSUM accumulation
  - finalize: one DVE scalar_tensor_tensor: out = psum * (1/sum) + a_fp32

Note: PE never executes fp32 ops — fp32 transpose-mode matmuls were
observed to hang the PE intermittently when interleaved with 16-bit
FWL-eligible matmul streams.
"""
import os
import sys
import types

import numpy as np


def _install_axon_hooks():
    """Provide antenv.axon_hooks (missing in this image) so trace=True works."""
    if 'antenv.axon_hooks' in sys.modules:
        return
    m = types.ModuleType('antenv.axon_hooks')
    m._hook = None
    m.set_axon_ntff_profile_hook = lambda h: setattr(m, '_hook', h)
    m.get_axon_ntff_profile_hook = lambda: m._hook
    sys.modules['antenv.axon_hooks'] = m
    try:
        import antenv
        antenv.axon_hooks = m
    except ImportError:
        pass
    try:
        from trn_agent_boot.trn_boot import _ntff_profile_via_ctypes
        m.set_axon_ntff_profile_hook(
            _ntff_profile_via_ctypes('/opt/axon/libaxon_pjrt.so'))
    except Exception:
        pass


_install_axon_hooks()

import concourse.bass as bass  # noqa: E402
import concourse.mybir as mybir  # noqa: E402
import concourse.tile as tile  # noqa: E402
from concourse import bacc, bass_utils  # noqa: E402
from concourse.masks import make_identity  # noqa: E402
from concourse.tile_rust import add_dep_helper  # noqa: E402

# artifact upload needs a bucket; keep everything local in the sandbox
bass_utils.upload_artifacts = lambda tmpdir: f"local:{tmpdir}"

N_CORES = 8
B, C, H, W = 16, 1024, 32, 32
HW = H * W
S = B // N_CORES        # samples per core
P = 128
NT = C // P             # 8 row tiles
F32 = mybir.dt.float32
F16 = mybir.dt.float16
ALU = mybir.AluOpType
AX = mybir.AxisListType
ACTF = mybir.ActivationFunctionType

# 1 = single-pass fp16 m1 (fast); 3 = fp16 hi/lo x3 m1 (high accuracy)
M1_TERMS = int(os.environ.get("CAM_M1_TERMS", "1"))


def cam_kernel(ctx, tc, out_ap, a_ap, b_ap, c_ap, n_samples=S):
    nc = tc.nc

    const_pool = ctx.enter_context(tc.tile_pool(name="const", bufs=1))
    big = ctx.enter_context(tc.tile_pool(name="big", bufs=1))
    big2 = ctx.enter_context(tc.tile_pool(name="big2", bufs=2))
    stg_pool = ctx.enter_context(tc.tile_pool(name="stage", bufs=3))
    epool = ctx.enter_context(tc.tile_pool(name="epool", bufs=2))
    opool = ctx.enter_context(tc.tile_pool(name="opool", bufs=2))
    arpool = ctx.enter_context(tc.tile_pool(name="ar", bufs=3))
    sm = ctx.enter_context(tc.tile_pool(name="sm", bufs=16))
    dram = ctx.enter_context(tc.tile_pool(name="dram", bufs=2, space="DRAM"))
    psum_t = ctx.enter_context(tc.tile_pool(name="psum_t", bufs=2, space="PSUM"))
    psum_s = ctx.enter_context(tc.tile_pool(name="psum_s", bufs=3, space="PSUM"))
    psum_o = ctx.enter_context(tc.tile_pool(name="psum_o", bufs=3, space="PSUM"))

    ident = const_pool.tile([P, P], F16)
    make_identity(nc, ident[:])

    for s in range(n_samples):
        a16 = big2.tile([P, NT, HW], F16, tag="a16")
        ET = big.tile([P, NT, C], F16, tag="ET")

        # ---- b,c -> fp16 transposed operands via cast-load + PE transpose ----
        # bcT free-dim layout (units of C): x1: [b, c]; x3: [b_hi, b_lo,
        # c_hi, c_lo]. m1_ops = (lhs col base, rhs col base) pairs.
        if M1_TERMS == 1:
            NW = 2
            bcT = big2.tile([P, NT, NW * C], F16, tag="bcT")
            srcs = [(b_ap, 0, None), (c_ap, C, None)]
            m1_ops = [(0, C)]
        else:
            NW = 4
            bcT = big.tile([P, NT, NW * C], F16, tag="bcT3")
            srcs = [(b_ap, 0, 1), (c_ap, 2 * C, 3)]
            m1_ops = [(0, 2 * C), (0, 3 * C), (C, 2 * C)]
        # order: b row 0, then all of c (m1 i=0 needs full cT), then b rest
        order = [(srcs[0], 0)] + [(srcs[1], r) for r in range(NT)] + \
                [(srcs[0], r) for r in range(1, NT)]
        for (src_ap, base, lo_q), r in order:
                rsl = slice(r * P, (r + 1) * P)
                nat = stg_pool.tile([P, HW], F16, tag="nat")
                nc.gpsimd.dma_start(nat[:], src_ap[s, rsl, :])  # cast f32->f16
                if lo_q is not None:
                    st = stg_pool.tile([P, HW], F32, tag="stage")
                    nc.sync.dma_start(st[:], src_ap[s, rsl, :])
                    lon = stg_pool.tile([P, HW], F16, tag="lon")
                    nc.vector.tensor_tensor(lon[:], st[:], nat[:],
                                            ALU.subtract)
                    nats = ((nat, base), (lon, base + C))
                else:
                    nats = ((nat, base),)
                for nt_, nb in nats:
                    pt = psum_t.tile([P, 8 * P], F16, tag="pt")
                    for j in range(NT):
                        nc.tensor.transpose(
                            pt[:, j * P:(j + 1) * P],
                            nt_[:, j * P:(j + 1) * P], ident[:])
                    nc.vector.tensor_copy(
                        bcT[:, :, nb + r * P:nb + (r + 1) * P],
                        pt[:].rearrange("p (t c) -> p t c", t=8))

        # ---- a -> fp16 via cast-DMA (m2 moving operand) ----
        for r in range(NT):
            nc.gpsimd.dma_start(a16[:, r, :], a_ap[s, r * P:(r + 1) * P, :])

        # ---- per output row-tile: m1, softmax, E^T, m2, finalize ----
        for i in range(NT):
            isl = slice(i * P, (i + 1) * P)
            ps0 = psum_s.tile([P, 512], F32, tag="ps")
            ps1 = psum_s.tile([P, 512], F32, tag="ps")
            n_acc = NT * len(m1_ops)
            acc = 0
            for kk in range(NT):
                for lo_, ro_ in m1_ops:
                    first = acc == 0
                    last = acc == n_acc - 1
                    lhsT = bcT[:, kk, lo_ + i * P:lo_ + (i + 1) * P]
                    nc.tensor.matmul(ps0[:], lhsT, bcT[:, kk, ro_:ro_ + 512],
                                     start=first, stop=last)
                    nc.tensor.matmul(ps1[:], lhsT,
                                     bcT[:, kk, ro_ + 512:ro_ + 1024],
                                     start=first, stop=last)
                    acc += 1

            m0 = sm.tile([P, 1], F32, tag="sc")
            m1t = sm.tile([P, 1], F32, tag="sc")
            nmx = sm.tile([P, 1], F32, tag="sc")
            nc.vector.tensor_reduce(m0[:], ps0[:], axis=AX.X, op=ALU.max)
            nc.vector.tensor_reduce(m1t[:], ps1[:], axis=AX.X, op=ALU.max)
            nc.vector.tensor_tensor(nmx[:], m0[:], m1t[:], ALU.max)
            nc.vector.tensor_scalar_mul(nmx[:], nmx[:], -1.0)

            E = epool.tile([P, C], F16, tag="E")
            rs0 = sm.tile([P, 1], F32, tag="sc")
            rs1 = sm.tile([P, 1], F32, tag="sc")
            nc.scalar.activation(E[:, 0:512], ps0[:], ACTF.Exp,
                                 bias=nmx[:], scale=1.0, accum_out=rs0[:])
            nc.scalar.activation(E[:, 512:1024], ps1[:], ACTF.Exp,
                                 bias=nmx[:], scale=1.0, accum_out=rs1[:])
            rinv = sm.tile([P, 1], F32, tag="sc")
            nc.vector.tensor_add(rinv[:], rs0[:], rs1[:])
            nc.vector.reciprocal(rinv[:], rinv[:])

            pt = psum_t.tile([P, 8 * P], F16, tag="pt")
            for j in range(8):
                nc.tensor.transpose(
                    pt[:, j * P:(j + 1) * P],
                    E[:, j * P:(j + 1) * P], ident[:])
            nc.vector.tensor_copy(
                ET[:, :, isl],
                pt[:].rearrange("p (t c) -> p t c", t=8))

            po0 = psum_o.tile([P, 512], F32, tag="po")
            po1 = psum_o.tile([P, 512], F32, tag="po")
            for jj in range(NT):
                first, last = jj == 0, jj == NT - 1
                l_e = ET[:, jj, isl]
                nc.tensor.matmul(po0[:], l_e, a16[:, jj, 0:512],
                                 start=first, stop=last)
                nc.tensor.matmul(po1[:], l_e, a16[:, jj, 512:1024],
                                 start=first, stop=last)

            ar = arpool.tile([P, HW], F32, tag="ar")
            nc.scalar.dma_start(ar[:], a_ap[s, isl, :])
            ot = opool.tile([P, HW], F32, tag="ot")
            nc.vector.scalar_tensor_tensor(
                ot[:, 0:512], po0[:], rinv[:], ar[:, 0:512],
                op0=ALU.mult, op1=ALU.add)
            nc.vector.scalar_tensor_tensor(
                ot[:, 512:1024], po1[:], rinv[:], ar[:, 512:1024],
                op0=ALU.mult, op1=ALU.add)
            nc.scalar.dma_start(out_ap[s, isl, :], ot[:])


_BUILT = {}


def build_program(n_samples=S):
    key = (M1_TERMS, n_samples)
    if key in _BUILT:
        return _BUILT[key]
    nc = bacc.Bacc("TRN2", target_bir_lowering=False, debug=False,
                   enable_asserts=False, num_devices=N_CORES)
    a = nc.dram_tensor("a", [S, C, HW], F32, kind="ExternalInput").ap()
    b = nc.dram_tensor("b", [S, C, HW], F32, kind="ExternalInput").ap()
    c = nc.dram_tensor("c", [S, C, HW], F32, kind="ExternalInput").ap()
    out = nc.dram_tensor("out", [S, C, HW], F32, kind="ExternalOutput").ap()
    from contextlib import ExitStack
    with tile.TileContext(nc) as tc, ExitStack() as ctx:
        cam_kernel(ctx, tc, out, a, b, c, n_samples=n_samples)
    nc.compile()
    _BUILT[key] = nc
    return nc


def run_sharded(a, b, c, trace=False, n_samples=S, **kw):
    """a,b,c: [16,1024,1024] fp32 -> (full output, BassKernelResults)."""
    nc = build_program(n_samples)
    in_maps = []
    for core in range(N_CORES):
        sl = slice(core * S, (core + 1) * S)
        in_maps.append({"a": np.ascontiguousarray(a[sl]),
                        "b": np.ascontiguousarray(b[sl]),
                        "c": np.ascontiguousarray(c[sl])})
    res = bass_utils.run_bass_kernel_spmd(
        nc, in_maps, core_ids=list(range(N_CORES)), trace=trace, **kw)
    out = np.concatenate([res.results[core]["out"] for core in range(N_CORES)],
                         axis=0)
    return out, res


def kernel(a, b, c):
    a = np.asarray(a, dtype=np.float32).reshape(B, C, HW)
    b = np.asarray(b, dtype=np.float32).reshape(B, C, HW)
    c = np.asarray(c, dtype=np.float32).reshape(B, C, HW)
    out, _ = run_sharded(a, b, c, trace=False)
    return out.reshape(B, C, H, W)


# revision 39
# speedup vs baseline: 1.4375x; 1.0724x over previous
"""Channel-attention (CAM) Trainium2 kernel.

Problem: out[b] = softmax(b_f[b] @ c_f[b].T, axis=-1) @ a_f[b] + a_f[b]
with a,b,c: [16, 1024, 32, 32] fp32, flattened to [16, 1024, 1024].

Sharding: pure data parallel over batch — 16 samples / 8 cores = 2 per core.

Per-core pipeline (per sample), fp16 compute:
  - b,c cast fp32->fp16 via DRAM->DRAM cast DMA, then DMA-transposed
    (XBAR) into [HW, C] operand layout (bT, cT)
  - m1: scores = bT.T @ cT, fp16 x1 (or x3 hi/lo split for high accuracy),
    fp32 PSUM accumulation
  - softmax: DVE row-max along free dim, ACT Exp with bias=-max and
    accum_out row-sum; the 1/sum division is deferred to the output
  - E (fp16) PE-transposed into ET (stationary operand of m2)
  - m2: out = ET.T @ a16, fp32 PSUM accumulation
  - finalize: one DVE scalar_tensor_tensor: out = psum * (1/sum) + a_fp32

Note: PE never executes fp32 ops — fp32 transpose-mode matmuls were
observed to hang the PE intermittently when interleaved with 16-bit
FWL-eligible matmul streams.
"""
import os
import sys
import types

import numpy as np


def _install_axon_hooks():
    """Provide antenv.axon_hooks (missing in this image) so trace=True works."""
    if 'antenv.axon_hooks' in sys.modules:
        return
    m = types.ModuleType('antenv.axon_hooks')
    m._hook = None
    m.set_axon_ntff_profile_hook = lambda h: setattr(m, '_hook', h)
    m.get_axon_ntff_profile_hook = lambda: m._hook
    sys.modules['antenv.axon_hooks'] = m
    try:
        import antenv
        antenv.axon_hooks = m
    except ImportError:
        pass
    try:
        from trn_agent_boot.trn_boot import _ntff_profile_via_ctypes
        m.set_axon_ntff_profile_hook(
            _ntff_profile_via_ctypes('/opt/axon/libaxon_pjrt.so'))
    except Exception:
        pass


_install_axon_hooks()

import concourse.bass as bass  # noqa: E402
import concourse.mybir as mybir  # noqa: E402
import concourse.tile as tile  # noqa: E402
from concourse import bacc, bass_utils  # noqa: E402
from concourse.masks import make_identity  # noqa: E402
from concourse.tile_rust import add_dep_helper  # noqa: E402

# artifact upload needs a bucket; keep everything local in the sandbox
bass_utils.upload_artifacts = lambda tmpdir: f"local:{tmpdir}"

N_CORES = 8
B, C, H, W = 16, 1024, 32, 32
HW = H * W
S = B // N_CORES        # samples per core
P = 128
NT = C // P             # 8 row tiles
F32 = mybir.dt.float32
F16 = mybir.dt.float16
ALU = mybir.AluOpType
AX = mybir.AxisListType
ACTF = mybir.ActivationFunctionType

# 1 = single-pass fp16 m1 (fast); 3 = fp16 hi/lo x3 m1 (high accuracy)
M1_TERMS = int(os.environ.get("CAM_M1_TERMS", "1"))


def cam_kernel(ctx, tc, out_ap, a_ap, b_ap, c_ap, n_samples=S):
    nc = tc.nc

    const_pool = ctx.enter_context(tc.tile_pool(name="const", bufs=1))
    big = ctx.enter_context(tc.tile_pool(name="big", bufs=1))
    big2 = ctx.enter_context(tc.tile_pool(name="big2", bufs=2))
    stg_pool = ctx.enter_context(tc.tile_pool(name="stage", bufs=4))
    epool = ctx.enter_context(tc.tile_pool(name="epool", bufs=2))
    opool = ctx.enter_context(tc.tile_pool(name="opool", bufs=2))
    arpool = ctx.enter_context(tc.tile_pool(name="ar", bufs=3))
    sm = ctx.enter_context(tc.tile_pool(name="sm", bufs=16))
    dram = ctx.enter_context(tc.tile_pool(name="dram", bufs=2, space="DRAM"))
    psum_t = ctx.enter_context(tc.tile_pool(name="psum_t", bufs=1, space="PSUM"))
    psum_s = ctx.enter_context(tc.tile_pool(name="psum_s", bufs=4, space="PSUM"))
    psum_o = ctx.enter_context(tc.tile_pool(name="psum_o", bufs=3, space="PSUM"))

    ident = const_pool.tile([P, P], F16)
    make_identity(nc, ident[:])

    for s in range(n_samples):
        a16 = big2.tile([P, NT, HW], F16, tag="a16")
        ET = big.tile([P, NT, C], F16, tag="ET")

        # ---- b,c -> fp16 transposed operands via cast-load + PE transpose ----
        # bcT free-dim layout (units of C): x1: [b, c]; x3: [b_hi, b_lo,
        # c_hi, c_lo]. m1_ops = (lhs col base, rhs col base) pairs.
        if M1_TERMS == 1:
            NW = 2
            bcT = big2.tile([P, NT, NW * C], F16, tag="bcT")
            srcs = [(b_ap, 0, None), (c_ap, C, None)]
            m1_ops = [(0, C)]
        else:
            NW = 4
            bcT = big.tile([P, NT, NW * C], F16, tag="bcT3")
            srcs = [(b_ap, 0, 1), (c_ap, 2 * C, 3)]
            m1_ops = [(0, 2 * C), (0, 3 * C), (C, 2 * C)]
        # order: b row 0, then all of c (m1 i=0 needs full cT), then b rest
        order = [(srcs[0], 0)] + [(srcs[1], r) for r in range(NT)] + \
                [(srcs[0], r) for r in range(1, NT)]
        for (src_ap, base, lo_q), r in order:
                rsl = slice(r * P, (r + 1) * P)
                nat = stg_pool.tile([P, HW], F16, tag="nat")
                nc.gpsimd.dma_start(nat[:], src_ap[s, rsl, :])  # cast f32->f16
                if lo_q is not None:
                    st = stg_pool.tile([P, HW], F32, tag="stage")
                    nc.sync.dma_start(st[:], src_ap[s, rsl, :])
                    lon = stg_pool.tile([P, HW], F16, tag="lon")
                    nc.vector.tensor_tensor(lon[:], st[:], nat[:],
                                            ALU.subtract)
                    nats = ((nat, base), (lon, base + C))
                else:
                    nats = ((nat, base),)
                for nt_, nb in nats:
                    pt = psum_t.tile([P, 8 * P], F16, tag="pt")
                    for j in range(NT):
                        nc.tensor.transpose(
                            pt[:, j * P:(j + 1) * P],
                            nt_[:, j * P:(j + 1) * P], ident[:])
                    nc.vector.tensor_copy(
                        bcT[:, :, nb + r * P:nb + (r + 1) * P],
                        pt[:].rearrange("p (t c) -> p t c", t=8))

        # ---- a -> fp16 via cast-DMA (m2 moving operand) ----
        for r in range(NT):
            nc.gpsimd.dma_start(a16[:, r, :], a_ap[s, r * P:(r + 1) * P, :])

        # ---- per output row-tile: m1, softmax, E^T, m2, finalize ----
        for i in range(NT):
            isl = slice(i * P, (i + 1) * P)
            ps0 = psum_s.tile([P, 512], F32, tag="ps")
            ps1 = psum_s.tile([P, 512], F32, tag="ps")
            n_acc = NT * len(m1_ops)
            acc = 0
            for kk in range(NT):
                for lo_, ro_ in m1_ops:
                    first = acc == 0
                    last = acc == n_acc - 1
                    lhsT = bcT[:, kk, lo_ + i * P:lo_ + (i + 1) * P]
                    nc.tensor.matmul(ps0[:], lhsT, bcT[:, kk, ro_:ro_ + 512],
                                     start=first, stop=last)
                    nc.tensor.matmul(ps1[:], lhsT,
                                     bcT[:, kk, ro_ + 512:ro_ + 1024],
                                     start=first, stop=last)
                    acc += 1

            m0 = sm.tile([P, 1], F32, tag="sc")
            m1t = sm.tile([P, 1], F32, tag="sc")
            nmx = sm.tile([P, 1], F32, tag="sc")
            nc.vector.tensor_reduce(m0[:], ps0[:], axis=AX.X, op=ALU.max)
            nc.vector.tensor_reduce(m1t[:], ps1[:], axis=AX.X, op=ALU.max)
            nc.vector.tensor_tensor(nmx[:], m0[:], m1t[:], ALU.max)
            nc.vector.tensor_scalar_mul(nmx[:], nmx[:], -1.0)

            E = epool.tile([P, C], F16, tag="E")
            rs0 = sm.tile([P, 1], F32, tag="sc")
            rs1 = sm.tile([P, 1], F32, tag="sc")
            nc.scalar.activation(E[:, 0:512], ps0[:], ACTF.Exp,
                                 bias=nmx[:], scale=1.0, accum_out=rs0[:])
            nc.scalar.activation(E[:, 512:1024], ps1[:], ACTF.Exp,
                                 bias=nmx[:], scale=1.0, accum_out=rs1[:])
            rinv = sm.tile([P, 1], F32, tag="sc")
            nc.vector.tensor_add(rinv[:], rs0[:], rs1[:])
            nc.vector.reciprocal(rinv[:], rinv[:])

            pt = psum_t.tile([P, 8 * P], F16, tag="pt")
            for j in range(8):
                nc.tensor.transpose(
                    pt[:, j * P:(j + 1) * P],
                    E[:, j * P:(j + 1) * P], ident[:])
            nc.vector.tensor_copy(
                ET[:, :, isl],
                pt[:].rearrange("p (t c) -> p t c", t=8))

            po0 = psum_o.tile([P, 512], F32, tag="po")
            po1 = psum_o.tile([P, 512], F32, tag="po")
            for jj in range(NT):
                first, last = jj == 0, jj == NT - 1
                l_e = ET[:, jj, isl]
                nc.tensor.matmul(po0[:], l_e, a16[:, jj, 0:512],
                                 start=first, stop=last)
                nc.tensor.matmul(po1[:], l_e, a16[:, jj, 512:1024],
                                 start=first, stop=last)

            ar = arpool.tile([P, HW], F32, tag="ar")
            nc.scalar.dma_start(ar[:], a_ap[s, isl, :])
            ot = opool.tile([P, HW], F32, tag="ot")
            nc.vector.scalar_tensor_tensor(
                ot[:, 0:512], po0[:], rinv[:], ar[:, 0:512],
                op0=ALU.mult, op1=ALU.add)
            nc.vector.scalar_tensor_tensor(
                ot[:, 512:1024], po1[:], rinv[:], ar[:, 512:1024],
                op0=ALU.mult, op1=ALU.add)
            nc.scalar.dma_start(out_ap[s, isl, :], ot[:])


_BUILT = {}


def build_program(n_samples=S):
    key = (M1_TERMS, n_samples)
    if key in _BUILT:
        return _BUILT[key]
    nc = bacc.Bacc("TRN2", target_bir_lowering=False, debug=False,
                   enable_asserts=False, num_devices=N_CORES)
    a = nc.dram_tensor("a", [S, C, HW], F32, kind="ExternalInput").ap()
    b = nc.dram_tensor("b", [S, C, HW], F32, kind="ExternalInput").ap()
    c = nc.dram_tensor("c", [S, C, HW], F32, kind="ExternalInput").ap()
    out = nc.dram_tensor("out", [S, C, HW], F32, kind="ExternalOutput").ap()
    from contextlib import ExitStack
    with tile.TileContext(nc) as tc, ExitStack() as ctx:
        cam_kernel(ctx, tc, out, a, b, c, n_samples=n_samples)
    nc.compile()
    _BUILT[key] = nc
    return nc


def run_sharded(a, b, c, trace=False, n_samples=S, **kw):
    """a,b,c: [16,1024,1024] fp32 -> (full output, BassKernelResults)."""
    nc = build_program(n_samples)
    in_maps = []
    for core in range(N_CORES):
        sl = slice(core * S, (core + 1) * S)
        in_maps.append({"a": np.ascontiguousarray(a[sl]),
                        "b": np.ascontiguousarray(b[sl]),
                        "c": np.ascontiguousarray(c[sl])})
    res = bass_utils.run_bass_kernel_spmd(
        nc, in_maps, core_ids=list(range(N_CORES)), trace=trace, **kw)
    out = np.concatenate([res.results[core]["out"] for core in range(N_CORES)],
                         axis=0)
    return out, res


def kernel(a, b, c):
    a = np.asarray(a, dtype=np.float32).reshape(B, C, HW)
    b = np.asarray(b, dtype=np.float32).reshape(B, C, HW)
    c = np.asarray(c, dtype=np.float32).reshape(B, C, HW)
    out, _ = run_sharded(a, b, c, trace=False)
    return out.reshape(B, C, H, W)
